# revision 25
# baseline (speedup 1.0000x reference)
"""Multi-head self-attention (B=4, T=2048, C=1024, H=16 heads, causal) on 8 TRN2
NeuronCores, head-tensor-parallel.

Per core c (owning heads 2c, 2c+1 = attn feature rows [c*128,(c+1)*128)):
  1. QKV (bf16 x / weights) per batch, interleaved with that batch's attention
     for q-blocks ib in {0,1,2} so ScalarE exp work overlaps the next batch's
     PE-dense QKV stream. qT/kT stay SBUF-resident bf16; PSUM evacuations run
     on DVE so the ACT queue carries only exps (keeps PE's transposes and
     score matmuls from stalling behind queued exps). v is PE-transposed to
     natural tiles; a whole-tile memset provides the denominator ones columns.
  2. Causal attention, software-pipelined: scores+exp for kv-tile jt+1 issue
     before the PV matmuls of jt, so PE never waits on ACT. Diagonal kv-tiles
     compute only the valid q-range; head 1's scores are written left-shifted
     so one exp covers exactly the valid columns of both heads; PV re-aligns
     head 1 on output. Only a 128-wide triangle block needs a mask (DVE).
  3. Normalization: reciprocal_approx_fast of the PV denominator row, DMA
     roundtrip broadcasts it across partitions, DVE multiply out of PSUM.
  4. Three AllToAll chunks: P0={ib0,ib1} (1MB), P1={ib2} (0.5MB), P2={ib3}
     (0.5MB). A2A0/A2A1 hide under pass-Y attention (ib3); A2A2 hides under
     the P0/P1 output projections; only proj P2 is a serial tail.
  5. Output projection per 512/256-token slices, + bias.
Host converts x/weights to bf16, gathers the 8 [1024 feat, 1024 tok] slices
and un-permutes tokens.
"""
import ml_dtypes
import numpy as np

import concourse.bass as bass
import concourse.tile as tile
from concourse import bacc, mybir
from concourse.bass_utils import run_bass_kernel_spmd

F32 = mybir.dt.float32
BF16 = mybir.dt.bfloat16

B, T, C = 4, 2048, 1024
N_HEADS, HEAD = 16, 64
N_CORES = 8
BT = B * T
TOK_PER_CORE = BT // N_CORES    # 1024
TB = 512                        # token block (matmul moving dim)
NKT = C // 128                  # 8 contraction tiles
SCALE = HEAD ** -0.5
W0, W1, W2 = 512, 256, 256      # per-destination tokens of the 3 A2A chunks


def build():
    nc = bacc.Bacc("TRN2", target_bir_lowering=False, debug=False, num_devices=N_CORES)

    xT = nc.dram_tensor("xT", [C, BT], BF16, kind="ExternalInput")
    wqkvT = nc.dram_tensor("wqkvT", [C, 384], BF16, kind="ExternalInput")
    wprojT = nc.dram_tensor("wprojT", [C, C], BF16, kind="ExternalInput")
    bmat = nc.dram_tensor("bmat", [128, 8], F32, kind="ExternalInput")
    tri_in = nc.dram_tensor("tri", [128, 128], BF16, kind="ExternalInput")
    ident_in = nc.dram_tensor("ident", [128, 128], BF16, kind="ExternalInput")

    outT = nc.dram_tensor("outT", [C, TOK_PER_CORE], F32, kind="ExternalOutput")

    rnorm_d = nc.dram_tensor("rnorm_d", [16, 2 * TB], F32)
    a2i0 = nc.dram_tensor("a2i0", [N_CORES, 128, W0], BF16)
    a2o0 = nc.dram_tensor("a2o0", [N_CORES, 128, W0], BF16)
    a2i1 = nc.dram_tensor("a2i1", [N_CORES, 128, W1], BF16)
    a2o1 = nc.dram_tensor("a2o1", [N_CORES, 128, W1], BF16)
    a2i2 = nc.dram_tensor("a2i2", [N_CORES, 128, W2], BF16)
    a2o2 = nc.dram_tensor("a2o2", [N_CORES, 128, W2], BF16)

    xT_r = xT.ap().rearrange("(kt p) n -> p kt n", p=128)

    with tile.TileContext(nc) as tc:
        with (
            tc.tile_pool(name="consts", bufs=1) as consts,
            tc.tile_pool(name="wp", bufs=1) as wp_pool,
            tc.tile_pool(name="xt", bufs=3) as xt_pool,
            tc.tile_pool(name="qk", bufs=4) as qk_pool,
            tc.tile_pool(name="vnat", bufs=4) as vnat_pool,
            tc.tile_pool(name="vte", bufs=2) as vte_pool,
            tc.tile_pool(name="exp", bufs=4) as exp_pool,
            tc.tile_pool(name="evac", bufs=2) as evac_pool,
            tc.tile_pool(name="sr", bufs=4) as sr_pool,
            tc.tile_pool(name="po", bufs=2) as po_pool,
            tc.tile_pool(name="ps_a", bufs=2, space="PSUM") as ps_a,  # qkv/transp/pv/proj
            tc.tile_pool(name="ps_b", bufs=2, space="PSUM") as ps_b,  # paired score tiles
        ):
            wqkv_sb = consts.tile([128, NKT, 384], BF16)
            nc.sync.dma_start(out=wqkv_sb, in_=wqkvT.ap().rearrange("(kt p) m -> p kt m", p=128))
            ident = consts.tile([128, 128], BF16)
            nc.sync.dma_start(out=ident, in_=ident_in.ap())
            tri_sb = consts.tile([128, 128], BF16)
            nc.sync.dma_start(out=tri_sb, in_=tri_in.ap())

            qTs, kTs, vns = [], [], []
            xt_tiles = {}

            # one DMA per xt tile: the 8 chained matmuls then carry a single
            # semaphore wait, letting back-to-back weight loads overlap
            def xt_prefetch(b, tb):
                if (b, tb) not in xt_tiles:
                    col0 = b * T + tb * TB
                    xt = xt_pool.tile([128, NKT, TB], BF16, tag="xt", name="xt")
                    nc.sync.dma_start(out=xt[:, :, :], in_=xT_r[:, :, col0:col0 + TB])
                    xt_tiles[(b, tb)] = xt

            def xt_load(b, tb):
                xt_prefetch(b, tb)
                return xt_tiles.pop((b, tb))

            def qkv_batch(b):
                tok0 = b * T
                qT = qk_pool.tile([128, T], BF16, tag="qT")
                kT = qk_pool.tile([128, T], BF16, tag="kT")
                # per-jt stride 256: [h0 64][ones][h1 64][ones][pad=1.0 ...] so
                # PV can use full-128-wide stationary tiles (cols 0:128, 65:193)
                v_nat = vnat_pool.tile([128, 16, 256], BF16, tag="vnat")
                qTs.append(qT); kTs.append(kT); vns.append(v_nat)
                nc.gpsimd.memset(v_nat[:, :, :], 1.0)

                pend_tr = []  # deferred (vte, tb) transposes, run behind next chain

                def flush_tr():
                    for vte, tb in pend_tr:
                        tr_ps = ps_a.tile([128, TB], BF16, tag="a", name="tr_ps")
                        for q in range(TB // 128):
                            jt = tb * 4 + q
                            qsl = slice(q * 128, (q + 1) * 128)
                            nc.tensor.transpose(tr_ps[:, qsl], vte[:, qsl], ident[:])
                            nc.vector.tensor_copy(
                                v_nat[:, jt, 0:64], tr_ps[:, q * 128:q * 128 + 64]
                            )
                            nc.vector.tensor_copy(
                                v_nat[:, jt, 65:129], tr_ps[:, q * 128 + 64:(q + 1) * 128]
                            )
                    pend_tr.clear()

                for tb in range(T // TB):
                    xt = xt_load(b, tb)
                    for m in range(3):  # 0=q, 1=k, 2=v (feature-major)
                        ps = ps_a.tile([128, TB], F32, tag="a")
                        for kt in range(NKT):
                            nc.tensor.matmul(
                                ps[:],
                                lhsT=wqkv_sb[:, kt, m * 128:(m + 1) * 128],
                                rhs=xt[:, kt, :],
                                start=(kt == 0),
                                stop=(kt == NKT - 1),
                            )
                        if m == 0:
                            flush_tr()
                        sl = slice(tb * TB, (tb + 1) * TB)
                        with nc.allow_low_precision(reason="qkv evac to bf16"):
                            if m == 0:
                                nc.vector.tensor_copy(qT[:, sl], ps[:])
                            elif m == 1:
                                nc.vector.tensor_copy(kT[:, sl], ps[:])
                            else:
                                vte = vte_pool.tile([128, TB], BF16, tag="vte")
                                nc.vector.tensor_copy(vte[:], ps[:])
                                pend_tr.append((vte, tb))
                flush_tr()

            # ---- software-pipelined causal attention over a list of blocks ----
            # blocks: (b, ib, blk_row, a2i_tensor, wdest, base_tok)
            def attn_run(blocks):
                pv_tiles = {}

                def score_exp(bi, jt):
                    b, ib = blocks[bi][0], blocks[bi][1]
                    diag = jt - ib * 4
                    off = 128 * diag if diag > 0 else 0
                    w = TB - off
                    qt = qTs[b][:, ib * TB:(ib + 1) * TB]
                    kT = kTs[b]
                    jsl = slice(jt * 128, (jt + 1) * 128)
                    s = ps_b.tile([128, 2 * TB], F32, tag="s")
                    # head0 at [off:TB]; head1 left-shifted to [TB:TB+w] so one
                    # exp covers [off : TB+w] with no gap
                    nc.tensor.matmul(
                        s[:, off:TB], lhsT=kT[0:64, jsl], rhs=qt[0:64, off:TB],
                        start=True, stop=True, tile_position=(0, 0),
                    )
                    nc.tensor.matmul(
                        s[:, TB:TB + w], lhsT=kT[64:128, jsl], rhs=qt[64:128, off:TB],
                        start=True, stop=True, tile_position=(64, 0),
                    )
                    e = exp_pool.tile([128, 2 * TB], BF16, tag="e")
                    nc.scalar.activation(
                        e[:, off:TB + w], s[:, off:TB + w],
                        mybir.ActivationFunctionType.Exp, scale=SCALE,
                    )
                    if diag >= 0:  # mask the 128-wide triangle sub-block per head
                        with nc.allow_low_precision(reason="exact 0/1 mask on bf16 probs"):
                            nc.vector.tensor_mul(
                                e[:, off:off + 128], e[:, off:off + 128], tri_sb[:]
                            )
                            nc.vector.tensor_mul(
                                e[:, TB:TB + 128], e[:, TB:TB + 128], tri_sb[:]
                            )
                    return e, off, w

                def pv_and_finish(entry):
                    (bi, jt, njt), (e, off, w) = entry
                    b = blocks[bi][0]
                    if jt == 0:
                        pv_tiles[bi] = ps_a.tile([128, 2 * TB], F32, tag="a", name="pv")
                    pv = pv_tiles[bi]
                    # full-128 stationary tiles; out rows 0:64 = head data,
                    # row 64 = denominator, rows 65:127 = junk (never read)
                    nc.tensor.matmul(
                        pv[:, off:TB], lhsT=vns[b][:, jt, 0:128], rhs=e[:, off:TB],
                        start=(jt == 0), stop=(jt == njt - 1),
                    )
                    nc.tensor.matmul(
                        pv[:, TB + off:2 * TB], lhsT=vns[b][:, jt, 65:193],
                        rhs=e[:, TB:TB + w],
                        start=(jt == 0), stop=(jt == njt - 1),
                    )
                    if jt == njt - 1:
                        norm_scatter(bi)

                def norm_scatter(bi):
                    _, _, blk, a2i_t, wdest, base_tok = blocks[bi]
                    pv = pv_tiles[bi]
                    srow = sr_pool.tile([1, 2 * TB], F32, tag="sr")
                    nc.vector.tensor_copy(srow[:], pv[64:65, :])
                    r32 = sr_pool.tile([1, 2 * TB], F32, tag="sr")
                    nc.vector.reciprocal_approx_fast(out=r32[:], in_=srow[:])
                    nc.sync.dma_start(out=rnorm_d.ap()[blk, :], in_=r32[:])
                    rb = evac_pool.tile([64, 2 * TB], F32, tag="rb")
                    base = rnorm_d.ap()[blk, :]
                    rb_src = bass.AP(
                        tensor=base.tensor,
                        offset=base.offset,
                        ap=[[0, 64]] + [list(p) for p in base.ap],
                    )
                    nc.sync.dma_start(out=rb[:], in_=rb_src)
                    outn = evac_pool.tile([64, 2 * TB], BF16, tag="on")
                    with nc.allow_low_precision(reason="normalized attn out as bf16"):
                        nc.vector.tensor_mul(outn[:], pv[0:64, :], rb[:])
                    t = 0
                    while t < TB:
                        g = base_tok + t
                        d = g // wdest
                        off_d = g - d * wdest
                        ln = min(wdest - off_d, TB - t)
                        nc.sync.dma_start(
                            out=a2i_t.ap()[d, 0:64, off_d:off_d + ln],
                            in_=outn[:, t:t + ln],
                        )
                        nc.sync.dma_start(
                            out=a2i_t.ap()[d, 64:128, off_d:off_d + ln],
                            in_=outn[:, TB + t:TB + t + ln],
                        )
                        t += ln

                items = []
                for bi, bs in enumerate(blocks):
                    njt = (bs[1] + 1) * 4
                    items += [(bi, jt, njt) for jt in range(njt)]
                pend = None
                for it in items:
                    cur = (it, score_exp(it[0], it[1]))
                    if pend is not None:
                        pv_and_finish(pend)
                    pend = cur
                pv_and_finish(pend)

            # ---- output projection for a gathered token slice ----
            def proj(at, w, col0_out):
                for dt in range(8):
                    ps = ps_a.tile([128, TB], F32, tag="a")
                    for kt in range(NKT):
                        nc.tensor.matmul(
                            ps[:, 0:w],
                            lhsT=wproj_sb[:, kt, dt * 128:(dt + 1) * 128],
                            rhs=at[:, kt, :],
                            start=(kt == 0),
                            stop=(kt == NKT - 1),
                        )
                    ot = po_pool.tile([128, TB], F32, tag="po")
                    nc.scalar.activation(
                        ot[:, 0:w], ps[:, 0:w],
                        mybir.ActivationFunctionType.Identity,
                        bias=bmat_sb[:, dt:dt + 1], scale=1.0,
                    )
                    nc.sync.dma_start(
                        out=outT.ap()[dt * 128:(dt + 1) * 128, col0_out:col0_out + w],
                        in_=ot[:, 0:w],
                    )

            # ---- pass X: QKV interleaved with attention for ib in {0,1} ----
            for b in range(B):
                qkv_batch(b)
                if b + 1 < B:  # prefetch next batch's first x tiles behind attn
                    xt_prefetch(b + 1, 0)
                    xt_prefetch(b + 1, 1)
                if b == 0:  # late big-constant loads keep startup DMAs lean
                    wproj_sb = wp_pool.tile([128, NKT, C], BF16)
                    nc.sync.dma_start(
                        out=wproj_sb,
                        in_=wprojT.ap().rearrange("(kt p) m -> p kt m", p=128),
                    )
                    bmat_sb = consts.tile([128, 8], F32)
                    nc.sync.dma_start(out=bmat_sb, in_=bmat.ap())
                attn_run([
                    (b, 0, 2 * b + 0, a2i0, W0, TB * (2 * b + 0)),
                    (b, 1, 2 * b + 1, a2i0, W0, TB * (2 * b + 1)),
                ])

            nc.gpsimd.collective_compute(
                "AllToAll", mybir.AluOpType.bypass,
                ins=[a2i0.ap()], outs=[a2o0.ap()],
                replica_groups=[list(range(N_CORES))],
            )

            # ---- ib=2 blocks cover A2A0 ----
            attn_run([(b, 2, 8 + b, a2i1, W1, TB * b) for b in range(B)])

            nc.gpsimd.collective_compute(
                "AllToAll", mybir.AluOpType.bypass,
                ins=[a2i1.ap()], outs=[a2o1.ap()],
                replica_groups=[list(range(N_CORES))],
            )

            # ---- pass Y (ib=3) covers A2A1 ----
            attn_run([(b, 3, 12 + b, a2i2, W2, TB * b) for b in range(B)])

            at0 = xt_pool.tile([128, NKT, W0], BF16, tag="at", bufs=2)
            for kt in range(NKT):
                nc.gpsimd.dma_start(out=at0[:, kt, :], in_=a2o0.ap()[kt, :, :])
            at1 = xt_pool.tile([128, NKT, W1], BF16, tag="at1", bufs=2)
            for kt in range(NKT):
                nc.gpsimd.dma_start(out=at1[:, kt, :], in_=a2o1.ap()[kt, :, :])

            nc.gpsimd.collective_compute(
                "AllToAll", mybir.AluOpType.bypass,
                ins=[a2i2.ap()], outs=[a2o2.ap()],
                replica_groups=[list(range(N_CORES))],
            )

            proj(at0, W0, 0)
            proj(at1, W1, W0)

            at2 = xt_pool.tile([128, NKT, W2], BF16, tag="at1", bufs=2)
            for kt in range(NKT):
                nc.gpsimd.dma_start(out=at2[:, kt, :], in_=a2o2.ap()[kt, :, :])
            proj(at2, W2, W0 + W1)

    nc.compile()
    return nc


_NC = None
_last_in_maps = None


def _get_nc():
    global _NC
    if _NC is None:
        _NC = build()
    return _NC


def _build_in_maps(x, w_qkv, w_proj, b_proj):
    x = np.asarray(x, dtype=np.float32)
    w_qkv = np.asarray(w_qkv, dtype=np.float32)
    w_proj = np.asarray(w_proj, dtype=np.float32)
    b_proj = np.asarray(b_proj, dtype=np.float32)

    xT = np.ascontiguousarray(x.reshape(BT, C).T).astype(ml_dtypes.bfloat16)
    wprojT = np.ascontiguousarray(w_proj.T).astype(ml_dtypes.bfloat16)
    bmat = np.ascontiguousarray(b_proj.reshape(8, 128).T)
    p = np.arange(128)[:, None]
    f = np.arange(128)[None, :]
    tri = (p <= f).astype(ml_dtypes.bfloat16)
    ident = np.eye(128, dtype=np.float32).astype(ml_dtypes.bfloat16)

    in_maps = []
    for c in range(N_CORES):
        rows = slice(c * 128, (c + 1) * 128)
        w_local = np.concatenate(
            [w_qkv[0:C][rows], w_qkv[C:2 * C][rows], w_qkv[2 * C:3 * C][rows]], axis=0
        )  # [384, C]
        in_maps.append({
            "xT": xT,
            "wqkvT": np.ascontiguousarray(w_local.T).astype(ml_dtypes.bfloat16),
            "wprojT": wprojT,
            "bmat": bmat,
            "tri": tri,
            "ident": ident,
        })
    return in_maps


def _token_of_col():
    """Global token index for each column of the concatenated outT buffers."""
    tok = np.empty(BT, dtype=np.int64)
    for c in range(N_CORES):
        base = c * TOK_PER_CORE
        for t in range(W0):  # P0: block (c//2, ib=c%2)
            tok[base + t] = T * (c // 2) + TB * (c % 2) + t
        for t in range(W1):  # P1: ib=2
            gy = W1 * c + t
            tok[base + W0 + t] = T * (gy // TB) + 2 * TB + gy % TB
        for t in range(W2):  # P2: ib=3
            gy = W2 * c + t
            tok[base + W0 + W1 + t] = T * (gy // TB) + 3 * TB + gy % TB
    return tok


def _unshard(outs):
    outT_full = np.concatenate(outs, axis=1)  # [C, BT] in per-core column order
    out = np.empty((C, BT), dtype=np.float32)
    out[:, _token_of_col()] = outT_full
    return np.ascontiguousarray(out.T).reshape(B, T, C)


def kernel(x, w_qkv, w_proj, b_proj):
    nc = _get_nc()
    in_maps = _build_in_maps(x, w_qkv, w_proj, b_proj)
    global _last_in_maps
    _last_in_maps = in_maps
    res = run_bass_kernel_spmd(nc, in_maps, core_ids=list(range(N_CORES)))
    return _unshard([res.results[c]["outT"] for c in range(N_CORES)])
